# revision 16
# baseline (speedup 1.0000x reference)
"""AGNN propagation kernel for 8 TRN2 NeuronCores.

Algorithm (matches reference):
    x_norm = x * rsqrt(sum(x^2, -1) + 1e-8)
    logit_e = beta * <x_norm[dst_e], x_norm[src_e]>        (in [-beta, beta])
    alpha_e = exp(logit_e) / (segsum_dst(exp(logit)) + 1e-8)
    out_i   = sum_{e: dst_e = i} alpha_e * x[src_e]

Because |logit| <= beta < 1, the segment-max subtraction in the reference is
numerically unnecessary (exp stays in [e^-1, e]); plain exp matches to ~1e-7.

Sharding: node-parallel, no collectives. Host sorts nodes by in-degree and
stripes them across the 8 cores (rank c, c+8, ...), so every core sees an
identical degree profile. Each core packs its nodes into blocks of 128
(1 node per SBUF partition); block b is padded to K_b = max degree in the
block (tight, because nodes are degree-sorted).

Per the sharding hint, the host also gathers the raw source features per
edge slot (pure index manipulation -- no arithmetic): xe[p, slot] =
[x[src] (32) | bias | 0] where bias is -1e20 for pad slots and 0 otherwise.
The device streams these tables SEQUENTIALLY (no indirect DMA, whose
128-descriptor-per-instruction SWDGE cost dominated previous versions) and
does all the math per edge: dot with the normalized dst vector (the bias
column rides along and sends pad logits to -inf), source-norm rsqrt,
exp via the scalar engine with fused segment-sum, weighted aggregation.

Device phases (per core, one SPMD graph):
  main: per group of blocks: sequential DMA of the edge payload tile,
        fused scalar_tensor_tensor dot + square, two free-dim reduces,
        exp+segsum on the scalar engine, weighted aggregation, output rows.
Host reassembles: out[node_order] = dense rows.
"""

import os
import numpy as np
from ml_dtypes import bfloat16

import concourse.bass as bass
import concourse.bacc as bacc
import concourse.mybir as mybir
import concourse.tile as tile

F32 = mybir.dt.float32
BF16 = mybir.dt.bfloat16
I32 = mybir.dt.int32


def _pin_act_tables():
    """Make Square/Ln/Exp all resolve to the one table set that contains
    all three (natural_log_exp_and_others), so the scalar engine never
    reloads activation tables inside the main loop. We mutate the cached
    act-table dict in place; set indices (= act_func_set_id) are unchanged.
    """
    from concourse.hw_specs import get_activation_tables
    import concourse._compat  # noqa: F401
    for arch in ("gen3",):
        try:
            tabs = get_activation_tables(arch)
        except Exception:
            continue
        AF = mybir.ActivationFunctionType
        keep = {AF.Square, AF.Ln, AF.Exp}
        if "natural_log_exp_and_others" not in tabs:
            continue
        if not (keep <= tabs["natural_log_exp_and_others"]):
            continue
        for name, s in tabs.items():
            if name != "natural_log_exp_and_others":
                s -= keep

N_CORES = 8
LAST_RESULT = None  # set by kernel() for profiling harnesses
P = 128          # SBUF partitions (= nodes per block)
D = 32           # feature dim
DW = 33          # payload row: x_src(32) + bias(1)
EPS = 1e-8
NEG_BIAS = -1e20  # pad-slot logit bias
CLAMP = -80.0    # lower clamp on beta*cos before exp (kills pads safely)
NB = 4           # blocks per DMA group

MULT = mybir.AluOpType.mult
ADD = mybir.AluOpType.add
MAXOP = mybir.AluOpType.max


# ----------------------------------------------------------------------------
# Host-side planning (index manipulation only; no FLOPs on tensor data)
# ----------------------------------------------------------------------------

def build_plan(edge_index: np.ndarray, n_nodes: int):
    src = np.asarray(edge_index[0], dtype=np.int64)
    dst = np.asarray(edge_index[1], dtype=np.int64)

    deg = np.bincount(dst, minlength=n_nodes).astype(np.int64)

    # Global degree-descending node order, striped over cores.
    order = np.argsort(-deg, kind="stable")

    nodes_per_core = (n_nodes + N_CORES - 1) // N_CORES
    blocks = (nodes_per_core + P - 1) // P
    slots = blocks * P                      # padded nodes per core

    # CSR of incoming edges (sorted by dst).
    eorder = np.argsort(dst, kind="stable")
    src_sorted = src[eorder]
    starts = np.zeros(n_nodes + 1, dtype=np.int64)
    np.cumsum(deg, out=starts[1:])

    # Shared per-block K: block b holds global ranks [b*P*NC, (b+1)*P*NC).
    deg_ranked = deg[order]
    K = np.zeros(blocks, dtype=np.int64)
    for b in range(blocks):
        lo = b * P * N_CORES
        hi = min(lo + P * N_CORES, n_nodes)
        K[b] = max(1, int(deg_ranked[lo:hi].max()) if hi > lo else 1)

    groups = [list(range(g, min(g + NB, blocks))) for g in range(0, blocks, NB)]
    offs = np.zeros(blocks + 1, dtype=np.int64)
    np.cumsum(K, out=offs[1:])
    totk = int(offs[-1])

    dummy = n_nodes  # pad rows reference this all-zero row

    # node_of[c, s]: global node id at core c, slot s (or -1 pad).
    ranks = np.arange(slots) * N_CORES  # slot -> global rank base
    node_of = np.full((N_CORES, slots), -1, dtype=np.int64)
    for c in range(N_CORES):
        r = ranks + c
        valid = r < n_nodes
        node_of[c, valid] = order[r[valid]]

    # Per-core gather index table [P, totk] and dst permutation [P, blocks].
    idx_all = np.full((N_CORES, P, totk), dummy, dtype=np.int64)
    perm = np.full((N_CORES, P, blocks), dummy, dtype=np.int64)
    for c in range(N_CORES):
        for b in range(blocks):
            kb = int(K[b])
            kk = np.arange(kb)[None, :]
            nd = node_of[c, b * P:(b + 1) * P]
            valid = nd >= 0
            ndv = np.where(valid, nd, 0)
            d_ = np.where(valid, deg[ndv], 0)
            perm[c, :, b] = np.where(valid, nd, dummy)
            take = kk < d_[:, None]
            p_ = np.where(take, starts[ndv][:, None] + kk, 0)
            idx_all[c, :, offs[b]:offs[b] + kb] = np.where(
                take, src_sorted[p_], dummy)

    return dict(
        n_nodes=n_nodes, blocks=blocks, slots=slots, groups=groups,
        K=K, offs=offs, totk=totk, dummy=dummy, node_of=node_of,
        idx_all=idx_all, perm=perm,
    )


# ----------------------------------------------------------------------------
# Bass graph builder (one SPMD graph shared by all cores)
# ----------------------------------------------------------------------------

def build_kernel(blocks: int, groups, K, offs, totk: int):
    nc = bacc.Bacc(None, target_bir_lowering=False, debug=False)

    xe_ext = nc.declare_dram_parameter("xe", [P, totk * DW], BF16, isOutput=False)
    xd_ext = nc.declare_dram_parameter("xd", [P, blocks * D], F32, isOutput=False)
    beta_ext = nc.declare_dram_parameter("beta", [P, 1], F32, isOutput=False)
    out_ext = nc.declare_dram_parameter("out", [blocks * P, D], F32, isOutput=True)

    with tile.TileContext(nc) as tc:
        with (
            tc.tile_pool(name="persist", bufs=1) as persist,
            tc.tile_pool(name="xin", bufs=2) as xin_pool,
            tc.tile_pool(name="scr", bufs=2) as scr_pool,
            tc.tile_pool(name="sqp", bufs=2) as sq_pool,
            tc.tile_pool(name="t2p", bufs=2) as t2_pool,
            tc.tile_pool(name="sm", bufs=3) as sm_pool,
            tc.tile_pool(name="outp", bufs=3) as out_pool,
        ):
            # ---- persistent small tiles -------------------------------------
            beta_sb = persist.tile([P, 1], F32)
            nc.sync.dma_start(out=beta_sb[:], in_=beta_ext[:, :])
            eps_sb = persist.tile([P, 1], F32)
            nc.vector.memset(eps_sb[:], EPS)

            # ---- normalize the block-node (dst) features --------------------
            # xd is the core's own nodes' raw features in block layout
            # [p, b, d] (host shard); normalize on-device, with a trailing
            # [1.0, 0.0] so the 34-wide dot picks up the bias field.
            xd_sb = persist.tile([P, blocks, D], F32)
            nc.sync.dma_start(out=xd_sb[:], in_=xd_ext[:, :])
            dsq = scr_pool.tile([P, blocks, D], F32, tag="dsq")
            nc.vector.scalar_tensor_tensor(
                out=dsq[:], in0=xd_sb[:], scalar=1.0, in1=xd_sb[:],
                op0=MULT, op1=MULT)
            dss = persist.tile([P, blocks], F32)
            nc.vector.tensor_reduce(out=dss[:], in_=dsq[:],
                                    axis=mybir.AxisListType.X, op=ADD)
            # dwinv = rsqrt(dss + eps) = exp(-0.5 * ln(dss + eps)); Ln, Exp
            # and Square share one activation table set -> no table reloads
            dw = persist.tile([P, blocks], F32)
            nc.scalar.activation(dw[:], dss[:],
                                 mybir.ActivationFunctionType.Ln,
                                 bias=eps_sb[:, :1])
            dwinv = persist.tile([P, blocks], F32)
            nc.scalar.activation(dwinv[:], dw[:],
                                 mybir.ActivationFunctionType.Exp,
                                 scale=-0.5)
            xnd_all = persist.tile([P, blocks, DW], BF16)
            nc.vector.scalar_tensor_tensor(
                out=xnd_all[:, :, 0:D], in0=xd_sb[:], scalar=1.0,
                in1=dwinv[:, :, None].to_broadcast([P, blocks, D]),
                op0=MULT, op1=MULT)
            nc.vector.memset(xnd_all[:, :, D:D + 1], 1.0)

            out_r = out_ext[:, :].rearrange("(b p) d -> b p d", p=P)
            xe_r = xe_ext[:, :].rearrange("p (s e) -> p s e", e=DW)

            # ---- main loop --------------------------------------------------
            for gi, grp in enumerate(groups):
                g0 = grp[0]
                nb = len(grp)
                o0, o1 = int(offs[g0]), int(offs[grp[-1] + 1])
                tk = o1 - o0

                xeg = xin_pool.tile([P, tk, DW], BF16, tag="xeg")
                nc.sync.dma_start(out=xeg[:], in_=xe_r[:, o0:o1, :])

                # t = xe * xnd (34 wide; bias col rides along), per block
                t = scr_pool.tile([P, tk, DW], BF16, tag="t")
                for j, b in enumerate(grp):
                    js = slice(int(offs[b]) - o0, int(offs[b + 1]) - o0)
                    nc.vector.scalar_tensor_tensor(
                        out=t[:, js, :], in0=xeg[:, js, :], scalar=1.0,
                        in1=xnd_all[:, b, None, :].to_broadcast(
                            [P, js.stop - js.start, DW]),
                        op0=MULT, op1=MULT)
                d0 = sm_pool.tile([P, tk], F32, tag="d0")
                nc.vector.tensor_reduce(out=d0[:], in_=t[:],
                                        axis=mybir.AxisListType.X, op=ADD)

                # ss = sum(xe^2): square on the scalar engine (frees DVE)
                sqt = sq_pool.tile([P, tk, D], BF16, tag="sqt")
                nc.scalar.activation(sqt[:], xeg[:, :, 0:D],
                                     mybir.ActivationFunctionType.Square)
                ss = sm_pool.tile([P, tk], F32, tag="ss")
                nc.vector.tensor_reduce(out=ss[:], in_=sqt[:],
                                        axis=mybir.AxisListType.X, op=ADD)
                # winv = rsqrt(ss + eps) via exp(-0.5 * ln(ss + eps))
                lns = sm_pool.tile([P, tk], F32, tag="lns")
                nc.scalar.activation(lns[:], ss[:],
                                     mybir.ActivationFunctionType.Ln,
                                     bias=eps_sb[:, :1])
                winv = sm_pool.tile([P, tk], F32, tag="winv")
                nc.scalar.activation(winv[:], lns[:],
                                     mybir.ActivationFunctionType.Exp,
                                     scale=-0.5)

                # logit = clamp(beta * d0 * winv, CLAMP); pads -> -inf-ish
                cosb = sm_pool.tile([P, tk], F32, tag="cosb")
                nc.vector.scalar_tensor_tensor(
                    out=cosb[:], in0=d0[:], scalar=beta_sb[:, :1],
                    in1=winv[:], op0=MULT, op1=MULT)
                nc.vector.tensor_scalar_max(cosb[:], cosb[:], CLAMP)

                # per-block exp + fused segment-sum on the scalar engine
                z = sm_pool.tile([P, tk], F32, tag="z")
                seg = sm_pool.tile([P, nb], F32, tag="seg")
                for j, b in enumerate(grp):
                    js = slice(int(offs[b]) - o0, int(offs[b + 1]) - o0)
                    nc.scalar.activation(
                        z[:, js], cosb[:, js],
                        mybir.ActivationFunctionType.Exp,
                        accum_out=seg[:, j:j + 1])

                # t2[p, d, k] = xe[p, k, d] * z[p, k]; reduce over k
                ov = out_pool.tile([P, nb, D], F32, tag="ov")
                kmax = max(int(K[b]) for b in grp)
                for j, b in enumerate(grp):
                    js = slice(int(offs[b]) - o0, int(offs[b + 1]) - o0)
                    kb = js.stop - js.start
                    t2f = t2_pool.tile([P, D, kmax], BF16, tag="t2")
                    t2 = t2f[:, :, 0:kb]
                    nc.vector.scalar_tensor_tensor(
                        out=t2.rearrange("p d k -> p k d"),
                        in0=xeg[:, js, 0:D], scalar=1.0,
                        in1=z[:, js, None].to_broadcast([P, kb, D]),
                        op0=MULT, op1=MULT)
                    nc.vector.tensor_reduce(out=ov[:, j, :], in_=t2,
                                            axis=mybir.AxisListType.X, op=ADD)

                # ov /= (seg + eps)
                nc.vector.tensor_scalar_add(seg[:], seg[:], EPS)
                rec = sm_pool.tile([P, nb], F32, tag="rec")
                nc.vector.reciprocal(rec[:], seg[:])
                ob = out_pool.tile([P, nb, D], F32, tag="ob")
                nc.vector.scalar_tensor_tensor(
                    out=ob[:], in0=ov[:], scalar=1.0,
                    in1=rec[:, :, None].to_broadcast([P, nb, D]),
                    op0=MULT, op1=MULT)
                for j, b in enumerate(grp):
                    nc.sync.dma_start(out=out_r[b], in_=ob[:, j, :])

    return nc


# ----------------------------------------------------------------------------
# Public entry point
# ----------------------------------------------------------------------------

def kernel(x: np.ndarray, beta: np.ndarray, edge_index: np.ndarray,
           _debug_sim: bool = False) -> np.ndarray:
    x = np.asarray(x, dtype=np.float32)
    beta = np.asarray(beta, dtype=np.float32)
    edge_index = np.asarray(edge_index)
    n_nodes, d_feat = x.shape
    assert d_feat == D

    plan = build_plan(edge_index, n_nodes)
    blocks, slots, totk = plan["blocks"], plan["slots"], plan["totk"]

    # base rows: [x (32) | bias | 0]; the dummy row is zero with bias -1e20
    base = np.zeros((n_nodes + 1, DW), dtype=np.float32)
    base[:n_nodes, 0:D] = x
    base[n_nodes, D] = NEG_BIAS

    _pin_act_tables()
    nc = build_kernel(blocks, plan["groups"], plan["K"], plan["offs"], totk)
    if not nc.is_finalized():
        nc.finalize()

    # per-core input maps (host gather = index manipulation only)
    in_maps = []
    beta_b = np.broadcast_to(beta.reshape(1, 1), (P, 1)).astype(np.float32).copy()
    for c in range(N_CORES):
        xe = base[plan["idx_all"][c]].astype(bfloat16)   # [P, totk, DW]
        xd = base[plan["perm"][c], 0:D]            # [P, blocks, D]
        in_maps.append({
            "xe": np.ascontiguousarray(xe.reshape(P, totk * DW)),
            "xd": np.ascontiguousarray(xd.reshape(P, blocks * D)),
            "beta": beta_b,
        })

    if _debug_sim:
        from concourse import bass_interp
        sim = bass_interp.MultiCoreSim(nc, N_CORES)
        for c in range(N_CORES):
            for k, vv in in_maps[c].items():
                sim.cores[c].tensor(k)[:] = vv
        sim.simulate()
        results = [{"out": sim.cores[c].mem_tensor("out").copy()}
                   for c in range(N_CORES)]
    else:
        from concourse.bass_utils import run_bass_kernel_spmd
        trace = bool(int(os.environ.get("AGNN_TRACE", "0")))
        tmpdir = os.environ.get("AGNN_TRACE_DIR") or None
        res = run_bass_kernel_spmd(nc, in_maps, core_ids=list(range(N_CORES)),
                                   trace=trace, tmpdir=tmpdir)
        results = res.results
        global LAST_RESULT
        LAST_RESULT = res

    out = np.zeros((n_nodes, d_feat), dtype=np.float32)
    node_of = plan["node_of"]
    for c in range(N_CORES):
        nd = node_of[c]
        valid = nd >= 0
        out[nd[valid]] = results[c]["out"][:slots][valid]
    return out


# revision 17
# speedup vs baseline: 1.3069x; 1.3069x over previous
"""AGNN propagation kernel for 8 TRN2 NeuronCores.

Algorithm (matches reference):
    x_norm = x * rsqrt(sum(x^2, -1) + 1e-8)
    logit_e = beta * <x_norm[dst_e], x_norm[src_e]>        (in [-beta, beta])
    alpha_e = exp(logit_e) / (segsum_dst(exp(logit)) + 1e-8)
    out_i   = sum_{e: dst_e = i} alpha_e * x[src_e]

Because |logit| <= beta < 1, the segment-max subtraction in the reference is
numerically unnecessary (exp stays in [e^-1, e]); plain exp matches to ~1e-7.

Sharding: node-parallel, no collectives. Host sorts nodes by in-degree and
stripes them across the 8 cores (rank c, c+8, ...), so every core sees an
identical degree profile. Each core packs its nodes into blocks of 128
(1 node per SBUF partition); block b is padded to K_b = max degree in the
block (tight, because nodes are degree-sorted).

Per the sharding hint, the host also gathers the raw source features per
edge slot (pure index manipulation -- no arithmetic): xe[p, slot] =
[x[src] (32) | bias | 0] where bias is -1e20 for pad slots and 0 otherwise.
The device streams these tables SEQUENTIALLY (no indirect DMA, whose
128-descriptor-per-instruction SWDGE cost dominated previous versions) and
does all the math per edge: dot with the normalized dst vector (the bias
column rides along and sends pad logits to -inf), source-norm rsqrt,
exp via the scalar engine with fused segment-sum, weighted aggregation.

Device phases (per core, one SPMD graph):
  main: per group of blocks: sequential DMA of the edge payload tile,
        fused scalar_tensor_tensor dot + square, two free-dim reduces,
        exp+segsum on the scalar engine, weighted aggregation, output rows.
Host reassembles: out[node_order] = dense rows.
"""

import os
import numpy as np
from ml_dtypes import bfloat16

import concourse.bass as bass
import concourse.bacc as bacc
import concourse.mybir as mybir
import concourse.tile as tile

F32 = mybir.dt.float32
BF16 = mybir.dt.bfloat16
I32 = mybir.dt.int32


def _pin_act_tables():
    """Make Square/Ln/Exp all resolve to the one table set that contains
    all three (natural_log_exp_and_others), so the scalar engine never
    reloads activation tables inside the main loop. We mutate the cached
    act-table dict in place; set indices (= act_func_set_id) are unchanged.
    """
    from concourse.hw_specs import get_activation_tables
    import concourse._compat  # noqa: F401
    for arch in ("gen3",):
        try:
            tabs = get_activation_tables(arch)
        except Exception:
            continue
        AF = mybir.ActivationFunctionType
        keep = {AF.Square, AF.Ln, AF.Exp}
        if "natural_log_exp_and_others" not in tabs:
            continue
        if not (keep <= tabs["natural_log_exp_and_others"]):
            continue
        for name, s in tabs.items():
            if name != "natural_log_exp_and_others":
                s -= keep

N_CORES = 8
LAST_RESULT = None  # set by kernel() for profiling harnesses
P = 128          # SBUF partitions (= nodes per block)
D = 32           # feature dim
DW = 33          # payload row: x_src(32) + bias(1)
EPS = 1e-8
NEG_BIAS = -1e20  # pad-slot logit bias
CLAMP = -80.0    # lower clamp on beta*cos before exp (kills pads safely)
NB = 4           # blocks per DMA group

MULT = mybir.AluOpType.mult
ADD = mybir.AluOpType.add
MAXOP = mybir.AluOpType.max


# ----------------------------------------------------------------------------
# Host-side planning (index manipulation only; no FLOPs on tensor data)
# ----------------------------------------------------------------------------

def build_plan(edge_index: np.ndarray, n_nodes: int):
    src = np.asarray(edge_index[0], dtype=np.int64)
    dst = np.asarray(edge_index[1], dtype=np.int64)

    deg = np.bincount(dst, minlength=n_nodes).astype(np.int64)

    # Global degree-descending node order, striped over cores.
    order = np.argsort(-deg, kind="stable")

    nodes_per_core = (n_nodes + N_CORES - 1) // N_CORES
    blocks = (nodes_per_core + P - 1) // P
    slots = blocks * P                      # padded nodes per core

    # CSR of incoming edges (sorted by dst).
    eorder = np.argsort(dst, kind="stable")
    src_sorted = src[eorder]
    starts = np.zeros(n_nodes + 1, dtype=np.int64)
    np.cumsum(deg, out=starts[1:])

    # Shared per-block K: block b holds global ranks [b*P*NC, (b+1)*P*NC).
    deg_ranked = deg[order]
    K = np.zeros(blocks, dtype=np.int64)
    for b in range(blocks):
        lo = b * P * N_CORES
        hi = min(lo + P * N_CORES, n_nodes)
        K[b] = max(1, int(deg_ranked[lo:hi].max()) if hi > lo else 1)

    groups = [list(range(g, min(g + NB, blocks))) for g in range(0, blocks, NB)]
    offs = np.zeros(blocks + 1, dtype=np.int64)
    np.cumsum(K, out=offs[1:])
    totk = int(offs[-1])

    dummy = n_nodes  # pad rows reference this all-zero row

    # node_of[c, s]: global node id at core c, slot s (or -1 pad).
    ranks = np.arange(slots) * N_CORES  # slot -> global rank base
    node_of = np.full((N_CORES, slots), -1, dtype=np.int64)
    for c in range(N_CORES):
        r = ranks + c
        valid = r < n_nodes
        node_of[c, valid] = order[r[valid]]

    # Per-core gather index table [P, totk] and dst permutation [P, blocks].
    idx_all = np.full((N_CORES, P, totk), dummy, dtype=np.int64)
    perm = np.full((N_CORES, P, blocks), dummy, dtype=np.int64)
    for c in range(N_CORES):
        for b in range(blocks):
            kb = int(K[b])
            kk = np.arange(kb)[None, :]
            nd = node_of[c, b * P:(b + 1) * P]
            valid = nd >= 0
            ndv = np.where(valid, nd, 0)
            d_ = np.where(valid, deg[ndv], 0)
            perm[c, :, b] = np.where(valid, nd, dummy)
            take = kk < d_[:, None]
            p_ = np.where(take, starts[ndv][:, None] + kk, 0)
            idx_all[c, :, offs[b]:offs[b] + kb] = np.where(
                take, src_sorted[p_], dummy)

    return dict(
        n_nodes=n_nodes, blocks=blocks, slots=slots, groups=groups,
        K=K, offs=offs, totk=totk, dummy=dummy, node_of=node_of,
        idx_all=idx_all, perm=perm,
    )


# ----------------------------------------------------------------------------
# Bass graph builder (one SPMD graph shared by all cores)
# ----------------------------------------------------------------------------

def build_kernel(blocks: int, groups, K, offs, totk: int):
    nc = bacc.Bacc(None, target_bir_lowering=False, debug=False)

    xe_ext = nc.declare_dram_parameter("xe", [P, totk * DW], F32, isOutput=False)
    xd_ext = nc.declare_dram_parameter("xd", [P, blocks * D], F32, isOutput=False)
    beta_ext = nc.declare_dram_parameter("beta", [P, 1], F32, isOutput=False)
    out_ext = nc.declare_dram_parameter("out", [blocks * P, D], F32, isOutput=True)

    with tile.TileContext(nc) as tc:
        with (
            tc.tile_pool(name="persist", bufs=1) as persist,
            tc.tile_pool(name="xin", bufs=2) as xin_pool,
            tc.tile_pool(name="scr", bufs=2) as scr_pool,
            tc.tile_pool(name="sqp", bufs=2) as sq_pool,
            tc.tile_pool(name="t2p", bufs=2) as t2_pool,
            tc.tile_pool(name="sm", bufs=3) as sm_pool,
            tc.tile_pool(name="outp", bufs=3) as out_pool,
        ):
            # ---- persistent small tiles -------------------------------------
            beta_sb = persist.tile([P, 1], F32)
            nc.sync.dma_start(out=beta_sb[:], in_=beta_ext[:, :])
            eps_sb = persist.tile([P, 1], F32)
            nc.vector.memset(eps_sb[:], EPS)

            # ---- normalize the block-node (dst) features --------------------
            # xd is the core's own nodes' raw features in block layout
            # [p, b, d] (host shard); normalize on-device, with a trailing
            # [1.0, 0.0] so the 34-wide dot picks up the bias field.
            xd_sb = persist.tile([P, blocks, D], F32)
            nc.sync.dma_start(out=xd_sb[:], in_=xd_ext[:, :])
            dsq = scr_pool.tile([P, blocks, D], F32, tag="dsq")
            nc.vector.scalar_tensor_tensor(
                out=dsq[:], in0=xd_sb[:], scalar=1.0, in1=xd_sb[:],
                op0=MULT, op1=MULT)
            dss = persist.tile([P, blocks], F32)
            nc.vector.tensor_reduce(out=dss[:], in_=dsq[:],
                                    axis=mybir.AxisListType.X, op=ADD)
            # dwinv = rsqrt(dss + eps) = exp(-0.5 * ln(dss + eps)); Ln, Exp
            # and Square share one activation table set -> no table reloads
            dw = persist.tile([P, blocks], F32)
            nc.scalar.activation(dw[:], dss[:],
                                 mybir.ActivationFunctionType.Ln,
                                 bias=eps_sb[:, :1])
            dwinv = persist.tile([P, blocks], F32)
            nc.scalar.activation(dwinv[:], dw[:],
                                 mybir.ActivationFunctionType.Exp,
                                 scale=-0.5)
            xnd_all = persist.tile([P, blocks, DW], F32)
            nc.vector.scalar_tensor_tensor(
                out=xnd_all[:, :, 0:D], in0=xd_sb[:], scalar=1.0,
                in1=dwinv[:, :, None].to_broadcast([P, blocks, D]),
                op0=MULT, op1=MULT)
            nc.vector.memset(xnd_all[:, :, D:D + 1], 1.0)

            out_r = out_ext[:, :].rearrange("(b p) d -> b p d", p=P)
            xe_r = xe_ext[:, :].rearrange("p (s e) -> p s e", e=DW)

            # ---- main loop --------------------------------------------------
            for gi, grp in enumerate(groups):
                g0 = grp[0]
                nb = len(grp)
                o0, o1 = int(offs[g0]), int(offs[grp[-1] + 1])
                tk = o1 - o0

                xeg = xin_pool.tile([P, tk, DW], F32, tag="xeg")
                nc.sync.dma_start(out=xeg[:], in_=xe_r[:, o0:o1, :])

                # t = xe * xnd (34 wide; bias col rides along), per block
                t = scr_pool.tile([P, tk, DW], F32, tag="t")
                for j, b in enumerate(grp):
                    js = slice(int(offs[b]) - o0, int(offs[b + 1]) - o0)
                    nc.vector.scalar_tensor_tensor(
                        out=t[:, js, :], in0=xeg[:, js, :], scalar=1.0,
                        in1=xnd_all[:, b, None, :].to_broadcast(
                            [P, js.stop - js.start, DW]),
                        op0=MULT, op1=MULT)
                d0 = sm_pool.tile([P, tk], F32, tag="d0")
                nc.vector.tensor_reduce(out=d0[:], in_=t[:],
                                        axis=mybir.AxisListType.X, op=ADD)

                # ss = sum(xe^2): square on the scalar engine (frees DVE)
                sqt = sq_pool.tile([P, tk, D], F32, tag="sqt")
                nc.scalar.activation(sqt[:], xeg[:, :, 0:D],
                                     mybir.ActivationFunctionType.Square)
                ss = sm_pool.tile([P, tk], F32, tag="ss")
                nc.vector.tensor_reduce(out=ss[:], in_=sqt[:],
                                        axis=mybir.AxisListType.X, op=ADD)
                # winv = rsqrt(ss + eps) via exp(-0.5 * ln(ss + eps))
                lns = sm_pool.tile([P, tk], F32, tag="lns")
                nc.scalar.activation(lns[:], ss[:],
                                     mybir.ActivationFunctionType.Ln,
                                     bias=eps_sb[:, :1])
                winv = sm_pool.tile([P, tk], F32, tag="winv")
                nc.scalar.activation(winv[:], lns[:],
                                     mybir.ActivationFunctionType.Exp,
                                     scale=-0.5)

                # logit = clamp(beta * d0 * winv, CLAMP); pads -> -inf-ish
                cosb = sm_pool.tile([P, tk], F32, tag="cosb")
                nc.vector.scalar_tensor_tensor(
                    out=cosb[:], in0=d0[:], scalar=beta_sb[:, :1],
                    in1=winv[:], op0=MULT, op1=MULT)
                nc.vector.tensor_scalar_max(cosb[:], cosb[:], CLAMP)

                # per-block exp + fused segment-sum on the scalar engine
                z = sm_pool.tile([P, tk], F32, tag="z")
                seg = sm_pool.tile([P, nb], F32, tag="seg")
                for j, b in enumerate(grp):
                    js = slice(int(offs[b]) - o0, int(offs[b + 1]) - o0)
                    nc.scalar.activation(
                        z[:, js], cosb[:, js],
                        mybir.ActivationFunctionType.Exp,
                        accum_out=seg[:, j:j + 1])

                # t2[p, d, k] = xe[p, k, d] * z[p, k]; reduce over k
                ov = out_pool.tile([P, nb, D], F32, tag="ov")
                kmax = max(int(K[b]) for b in grp)
                for j, b in enumerate(grp):
                    js = slice(int(offs[b]) - o0, int(offs[b + 1]) - o0)
                    kb = js.stop - js.start
                    t2f = t2_pool.tile([P, D, kmax], F32, tag="t2")
                    t2 = t2f[:, :, 0:kb]
                    nc.vector.scalar_tensor_tensor(
                        out=t2.rearrange("p d k -> p k d"),
                        in0=xeg[:, js, 0:D], scalar=1.0,
                        in1=z[:, js, None].to_broadcast([P, kb, D]),
                        op0=MULT, op1=MULT)
                    nc.vector.tensor_reduce(out=ov[:, j, :], in_=t2,
                                            axis=mybir.AxisListType.X, op=ADD)

                # ov /= (seg + eps)
                nc.vector.tensor_scalar_add(seg[:], seg[:], EPS)
                rec = sm_pool.tile([P, nb], F32, tag="rec")
                nc.vector.reciprocal(rec[:], seg[:])
                ob = out_pool.tile([P, nb, D], F32, tag="ob")
                nc.vector.scalar_tensor_tensor(
                    out=ob[:], in0=ov[:], scalar=1.0,
                    in1=rec[:, :, None].to_broadcast([P, nb, D]),
                    op0=MULT, op1=MULT)
                for j, b in enumerate(grp):
                    nc.sync.dma_start(out=out_r[b], in_=ob[:, j, :])

    return nc


# ----------------------------------------------------------------------------
# Public entry point
# ----------------------------------------------------------------------------

def kernel(x: np.ndarray, beta: np.ndarray, edge_index: np.ndarray,
           _debug_sim: bool = False) -> np.ndarray:
    x = np.asarray(x, dtype=np.float32)
    beta = np.asarray(beta, dtype=np.float32)
    edge_index = np.asarray(edge_index)
    n_nodes, d_feat = x.shape
    assert d_feat == D

    plan = build_plan(edge_index, n_nodes)
    blocks, slots, totk = plan["blocks"], plan["slots"], plan["totk"]

    # base rows: [x (32) | bias | 0]; the dummy row is zero with bias -1e20
    base = np.zeros((n_nodes + 1, DW), dtype=np.float32)
    base[:n_nodes, 0:D] = x
    base[n_nodes, D] = NEG_BIAS

    _pin_act_tables()
    nc = build_kernel(blocks, plan["groups"], plan["K"], plan["offs"], totk)
    if not nc.is_finalized():
        nc.finalize()

    # per-core input maps (host gather = index manipulation only)
    in_maps = []
    beta_b = np.broadcast_to(beta.reshape(1, 1), (P, 1)).astype(np.float32).copy()
    for c in range(N_CORES):
        xe = base[plan["idx_all"][c]]              # [P, totk, DW]
        xd = base[plan["perm"][c], 0:D]            # [P, blocks, D]
        in_maps.append({
            "xe": np.ascontiguousarray(xe.reshape(P, totk * DW)),
            "xd": np.ascontiguousarray(xd.reshape(P, blocks * D)),
            "beta": beta_b,
        })

    if _debug_sim:
        from concourse import bass_interp
        sim = bass_interp.MultiCoreSim(nc, N_CORES)
        for c in range(N_CORES):
            for k, vv in in_maps[c].items():
                sim.cores[c].tensor(k)[:] = vv
        sim.simulate()
        results = [{"out": sim.cores[c].mem_tensor("out").copy()}
                   for c in range(N_CORES)]
    else:
        from concourse.bass_utils import run_bass_kernel_spmd
        trace = bool(int(os.environ.get("AGNN_TRACE", "0")))
        tmpdir = os.environ.get("AGNN_TRACE_DIR") or None
        res = run_bass_kernel_spmd(nc, in_maps, core_ids=list(range(N_CORES)),
                                   trace=trace, tmpdir=tmpdir)
        results = res.results
        global LAST_RESULT
        LAST_RESULT = res

    out = np.zeros((n_nodes, d_feat), dtype=np.float32)
    node_of = plan["node_of"]
    for c in range(N_CORES):
        nd = node_of[c]
        valid = nd >= 0
        out[nd[valid]] = results[c]["out"][:slots][valid]
    return out


# revision 18
# speedup vs baseline: 1.4455x; 1.1061x over previous
"""AGNN propagation kernel for 8 TRN2 NeuronCores.

Algorithm (matches reference):
    x_norm = x * rsqrt(sum(x^2, -1) + 1e-8)
    logit_e = beta * <x_norm[dst_e], x_norm[src_e]>        (in [-beta, beta])
    alpha_e = exp(logit_e) / (segsum_dst(exp(logit)) + 1e-8)
    out_i   = sum_{e: dst_e = i} alpha_e * x[src_e]

Because |logit| <= beta < 1, the segment-max subtraction in the reference is
numerically unnecessary (exp stays in [e^-1, e]); plain exp matches to ~1e-7.

Sharding: node-parallel, no collectives. Host sorts nodes by in-degree and
stripes them across the 8 cores (rank c, c+8, ...), so every core sees an
identical degree profile. Each core packs its nodes into blocks of 128
(1 node per SBUF partition); block b is padded to K_b = max degree in the
block (tight, because nodes are degree-sorted).

Per the sharding hint, the host also gathers the raw source features per
edge slot (pure index manipulation -- no arithmetic): xe[p, slot] =
[x[src] (32) | bias | 0] where bias is -1e20 for pad slots and 0 otherwise.
The device streams these tables SEQUENTIALLY (no indirect DMA, whose
128-descriptor-per-instruction SWDGE cost dominated previous versions) and
does all the math per edge: dot with the normalized dst vector (the bias
column rides along and sends pad logits to -inf), source-norm rsqrt,
exp via the scalar engine with fused segment-sum, weighted aggregation.

Device phases (per core, one SPMD graph):
  main: per group of blocks: sequential DMA of the edge payload tile,
        fused scalar_tensor_tensor dot + square, two free-dim reduces,
        exp+segsum on the scalar engine, weighted aggregation, output rows.
Host reassembles: out[node_order] = dense rows.
"""

import os
import numpy as np
from ml_dtypes import bfloat16

import concourse.bass as bass
import concourse.bacc as bacc
import concourse.mybir as mybir
import concourse.tile as tile

F32 = mybir.dt.float32
BF16 = mybir.dt.bfloat16
I32 = mybir.dt.int32


def _pin_act_tables():
    """Make Square/Ln/Exp all resolve to the one table set that contains
    all three (natural_log_exp_and_others), so the scalar engine never
    reloads activation tables inside the main loop. We mutate the cached
    act-table dict in place; set indices (= act_func_set_id) are unchanged.
    """
    from concourse.hw_specs import get_activation_tables
    import concourse._compat  # noqa: F401
    for arch in ("gen3",):
        try:
            tabs = get_activation_tables(arch)
        except Exception:
            continue
        AF = mybir.ActivationFunctionType
        keep = {AF.Square, AF.Ln, AF.Exp}
        if "natural_log_exp_and_others" not in tabs:
            continue
        if not (keep <= tabs["natural_log_exp_and_others"]):
            continue
        for name, s in tabs.items():
            if name != "natural_log_exp_and_others":
                s -= keep

N_CORES = 8
LAST_RESULT = None  # set by kernel() for profiling harnesses
P = 128          # SBUF partitions (= nodes per block)
D = 32           # feature dim
DW = 33          # payload row: x_src(32) + bias(1)
EPS = 1e-8
NEG_BIAS = -1e20  # pad-slot logit bias
CLAMP = -80.0    # lower clamp on beta*cos before exp (kills pads safely)
NB = 4           # blocks per DMA group

MULT = mybir.AluOpType.mult
ADD = mybir.AluOpType.add
MAXOP = mybir.AluOpType.max


# ----------------------------------------------------------------------------
# Host-side planning (index manipulation only; no FLOPs on tensor data)
# ----------------------------------------------------------------------------

def build_plan(edge_index: np.ndarray, n_nodes: int):
    src = np.asarray(edge_index[0], dtype=np.int64)
    dst = np.asarray(edge_index[1], dtype=np.int64)

    deg = np.bincount(dst, minlength=n_nodes).astype(np.int64)

    # Global degree-descending node order, striped over cores.
    order = np.argsort(-deg, kind="stable")

    nodes_per_core = (n_nodes + N_CORES - 1) // N_CORES
    blocks = (nodes_per_core + P - 1) // P
    slots = blocks * P                      # padded nodes per core

    # CSR of incoming edges (sorted by dst).
    eorder = np.argsort(dst, kind="stable")
    src_sorted = src[eorder]
    starts = np.zeros(n_nodes + 1, dtype=np.int64)
    np.cumsum(deg, out=starts[1:])

    # Shared per-block K: block b holds global ranks [b*P*NC, (b+1)*P*NC).
    deg_ranked = deg[order]
    K = np.zeros(blocks, dtype=np.int64)
    for b in range(blocks):
        lo = b * P * N_CORES
        hi = min(lo + P * N_CORES, n_nodes)
        K[b] = max(1, int(deg_ranked[lo:hi].max()) if hi > lo else 1)

    groups = [list(range(g, min(g + NB, blocks))) for g in range(0, blocks, NB)]
    offs = np.zeros(blocks + 1, dtype=np.int64)
    np.cumsum(K, out=offs[1:])
    totk = int(offs[-1])

    dummy = n_nodes  # pad rows reference this all-zero row

    # node_of[c, s]: global node id at core c, slot s (or -1 pad).
    ranks = np.arange(slots) * N_CORES  # slot -> global rank base
    node_of = np.full((N_CORES, slots), -1, dtype=np.int64)
    for c in range(N_CORES):
        r = ranks + c
        valid = r < n_nodes
        node_of[c, valid] = order[r[valid]]

    # Per-core gather index table [P, totk] and dst permutation [P, blocks].
    idx_all = np.full((N_CORES, P, totk), dummy, dtype=np.int64)
    perm = np.full((N_CORES, P, blocks), dummy, dtype=np.int64)
    for c in range(N_CORES):
        for b in range(blocks):
            kb = int(K[b])
            kk = np.arange(kb)[None, :]
            nd = node_of[c, b * P:(b + 1) * P]
            valid = nd >= 0
            ndv = np.where(valid, nd, 0)
            d_ = np.where(valid, deg[ndv], 0)
            perm[c, :, b] = np.where(valid, nd, dummy)
            take = kk < d_[:, None]
            p_ = np.where(take, starts[ndv][:, None] + kk, 0)
            idx_all[c, :, offs[b]:offs[b] + kb] = np.where(
                take, src_sorted[p_], dummy)

    return dict(
        n_nodes=n_nodes, blocks=blocks, slots=slots, groups=groups,
        K=K, offs=offs, totk=totk, dummy=dummy, node_of=node_of,
        idx_all=idx_all, perm=perm,
    )


# ----------------------------------------------------------------------------
# Bass graph builder (one SPMD graph shared by all cores)
# ----------------------------------------------------------------------------

def build_kernel(blocks: int, groups, K, offs, totk: int):
    nc = bacc.Bacc(None, target_bir_lowering=False, debug=False)

    xe_ext = nc.declare_dram_parameter("xe", [P, totk * DW], F32, isOutput=False)
    xd_ext = nc.declare_dram_parameter("xd", [P, blocks * D], F32, isOutput=False)
    beta_ext = nc.declare_dram_parameter("beta", [P, 1], F32, isOutput=False)
    out_ext = nc.declare_dram_parameter("out", [blocks * P, D], F32, isOutput=True)

    with tile.TileContext(nc) as tc:
        with (
            tc.tile_pool(name="persist", bufs=1) as persist,
            tc.tile_pool(name="xin", bufs=2) as xin_pool,
            tc.tile_pool(name="scr", bufs=2) as scr_pool,
            tc.tile_pool(name="sqp", bufs=2) as sq_pool,
            tc.tile_pool(name="t2p", bufs=2) as t2_pool,
            tc.tile_pool(name="sm", bufs=3) as sm_pool,
            tc.tile_pool(name="outp", bufs=3) as out_pool,
        ):
            # ---- persistent small tiles -------------------------------------
            beta_sb = persist.tile([P, 1], F32)
            nc.sync.dma_start(out=beta_sb[:], in_=beta_ext[:, :])
            eps_sb = persist.tile([P, 1], F32)
            nc.vector.memset(eps_sb[:], EPS)

            # ---- normalize the block-node (dst) features --------------------
            # xd is the core's own nodes' raw features in block layout
            # [p, b, d] (host shard); normalize on-device, with a trailing
            # [1.0, 0.0] so the 34-wide dot picks up the bias field.
            xd_sb = persist.tile([P, blocks, D], F32)
            nc.sync.dma_start(out=xd_sb[:], in_=xd_ext[:, :])
            dsq = scr_pool.tile([P, blocks, D], F32, tag="dsq")
            nc.vector.scalar_tensor_tensor(
                out=dsq[:], in0=xd_sb[:], scalar=1.0, in1=xd_sb[:],
                op0=MULT, op1=MULT)
            dss = persist.tile([P, blocks], F32)
            nc.vector.tensor_reduce(out=dss[:], in_=dsq[:],
                                    axis=mybir.AxisListType.X, op=ADD)
            # dwinv = rsqrt(dss + eps) = exp(-0.5 * ln(dss + eps)); Ln, Exp
            # and Square share one activation table set -> no table reloads
            dw = persist.tile([P, blocks], F32)
            nc.scalar.activation(dw[:], dss[:],
                                 mybir.ActivationFunctionType.Ln,
                                 bias=eps_sb[:, :1])
            dwinv = persist.tile([P, blocks], F32)
            nc.scalar.activation(dwinv[:], dw[:],
                                 mybir.ActivationFunctionType.Exp,
                                 scale=-0.5)
            xnd_all = persist.tile([P, blocks, DW], F32)
            nc.vector.scalar_tensor_tensor(
                out=xnd_all[:, :, 0:D], in0=xd_sb[:], scalar=1.0,
                in1=dwinv[:, :, None].to_broadcast([P, blocks, D]),
                op0=MULT, op1=MULT)
            nc.vector.memset(xnd_all[:, :, D:D + 1], 1.0)

            out_r = out_ext[:, :].rearrange("(b p) d -> b p d", p=P)
            xe_r = xe_ext[:, :].rearrange("p (s e) -> p s e", e=DW)

            # ---- main loop --------------------------------------------------
            for gi, grp in enumerate(groups):
                g0 = grp[0]
                nb = len(grp)
                o0, o1 = int(offs[g0]), int(offs[grp[-1] + 1])
                tk = o1 - o0

                xeg = xin_pool.tile([P, tk, DW], F32, tag="xeg")
                nc.sync.dma_start(out=xeg[:], in_=xe_r[:, o0:o1, :])

                # t = xe * xnd (34 wide; bias col rides along), per block
                t = scr_pool.tile([P, tk, DW], F32, tag="t")
                for j, b in enumerate(grp):
                    js = slice(int(offs[b]) - o0, int(offs[b + 1]) - o0)
                    nc.vector.scalar_tensor_tensor(
                        out=t[:, js, :], in0=xeg[:, js, :], scalar=1.0,
                        in1=xnd_all[:, b, None, :].to_broadcast(
                            [P, js.stop - js.start, DW]),
                        op0=MULT, op1=MULT)
                d0 = sm_pool.tile([P, tk], F32, tag="d0")
                nc.vector.tensor_reduce(out=d0[:], in_=t[:],
                                        axis=mybir.AxisListType.X, op=ADD)

                # ss = sum(xe^2): square on the scalar engine (frees DVE)
                sqt = sq_pool.tile([P, tk, D], F32, tag="sqt")
                nc.scalar.activation(sqt[:], xeg[:, :, 0:D],
                                     mybir.ActivationFunctionType.Square)
                ss = sm_pool.tile([P, tk], F32, tag="ss")
                nc.vector.tensor_reduce(out=ss[:], in_=sqt[:],
                                        axis=mybir.AxisListType.X, op=ADD)
                # winv = rsqrt(ss + eps) via exp(-0.5 * ln(ss + eps))
                lns = sm_pool.tile([P, tk], F32, tag="lns")
                nc.scalar.activation(lns[:], ss[:],
                                     mybir.ActivationFunctionType.Ln,
                                     bias=eps_sb[:, :1])
                winv = sm_pool.tile([P, tk], F32, tag="winv")
                nc.scalar.activation(winv[:], lns[:],
                                     mybir.ActivationFunctionType.Exp,
                                     scale=-0.5)

                # logit = clamp(beta * d0 * winv, CLAMP); pads -> -inf-ish
                cosb = sm_pool.tile([P, tk], F32, tag="cosb")
                nc.vector.scalar_tensor_tensor(
                    out=cosb[:], in0=d0[:], scalar=beta_sb[:, :1],
                    in1=winv[:], op0=MULT, op1=MULT)
                nc.vector.tensor_scalar_max(cosb[:], cosb[:], CLAMP)

                # per-block exp + fused segment-sum on the scalar engine
                z = sm_pool.tile([P, tk], F32, tag="z")
                seg = sm_pool.tile([P, nb], F32, tag="seg")
                for j, b in enumerate(grp):
                    js = slice(int(offs[b]) - o0, int(offs[b + 1]) - o0)
                    nc.scalar.activation(
                        z[:, js], cosb[:, js],
                        mybir.ActivationFunctionType.Exp,
                        accum_out=seg[:, j:j + 1])

                # t2[p, d, k] = xe[p, k, d] * z[p, k]; reduce over k
                ov = out_pool.tile([P, nb, D], F32, tag="ov")
                kmax = max(int(K[b]) for b in grp)
                for j, b in enumerate(grp):
                    js = slice(int(offs[b]) - o0, int(offs[b + 1]) - o0)
                    kb = js.stop - js.start
                    t2f = t2_pool.tile([P, D, kmax], F32, tag="t2")
                    t2 = t2f[:, :, 0:kb]
                    nc.gpsimd.tensor_tensor(
                        out=t2.rearrange("p d k -> p k d"),
                        in0=xeg[:, js, 0:D],
                        in1=z[:, js, None].to_broadcast([P, kb, D]),
                        op=MULT)
                    nc.vector.tensor_reduce(out=ov[:, j, :], in_=t2,
                                            axis=mybir.AxisListType.X, op=ADD)

                # ov /= (seg + eps)
                nc.vector.tensor_scalar_add(seg[:], seg[:], EPS)
                rec = sm_pool.tile([P, nb], F32, tag="rec")
                nc.vector.reciprocal(rec[:], seg[:])
                ob = out_pool.tile([P, nb, D], F32, tag="ob")
                nc.vector.scalar_tensor_tensor(
                    out=ob[:], in0=ov[:], scalar=1.0,
                    in1=rec[:, :, None].to_broadcast([P, nb, D]),
                    op0=MULT, op1=MULT)
                for j, b in enumerate(grp):
                    nc.sync.dma_start(out=out_r[b], in_=ob[:, j, :])

    return nc


# ----------------------------------------------------------------------------
# Public entry point
# ----------------------------------------------------------------------------

def kernel(x: np.ndarray, beta: np.ndarray, edge_index: np.ndarray,
           _debug_sim: bool = False) -> np.ndarray:
    x = np.asarray(x, dtype=np.float32)
    beta = np.asarray(beta, dtype=np.float32)
    edge_index = np.asarray(edge_index)
    n_nodes, d_feat = x.shape
    assert d_feat == D

    plan = build_plan(edge_index, n_nodes)
    blocks, slots, totk = plan["blocks"], plan["slots"], plan["totk"]

    # base rows: [x (32) | bias | 0]; the dummy row is zero with bias -1e20
    base = np.zeros((n_nodes + 1, DW), dtype=np.float32)
    base[:n_nodes, 0:D] = x
    base[n_nodes, D] = NEG_BIAS

    _pin_act_tables()
    nc = build_kernel(blocks, plan["groups"], plan["K"], plan["offs"], totk)
    if not nc.is_finalized():
        nc.finalize()

    # per-core input maps (host gather = index manipulation only)
    in_maps = []
    beta_b = np.broadcast_to(beta.reshape(1, 1), (P, 1)).astype(np.float32).copy()
    for c in range(N_CORES):
        xe = base[plan["idx_all"][c]]              # [P, totk, DW]
        xd = base[plan["perm"][c], 0:D]            # [P, blocks, D]
        in_maps.append({
            "xe": np.ascontiguousarray(xe.reshape(P, totk * DW)),
            "xd": np.ascontiguousarray(xd.reshape(P, blocks * D)),
            "beta": beta_b,
        })

    if _debug_sim:
        from concourse import bass_interp
        sim = bass_interp.MultiCoreSim(nc, N_CORES)
        for c in range(N_CORES):
            for k, vv in in_maps[c].items():
                sim.cores[c].tensor(k)[:] = vv
        sim.simulate()
        results = [{"out": sim.cores[c].mem_tensor("out").copy()}
                   for c in range(N_CORES)]
    else:
        from concourse.bass_utils import run_bass_kernel_spmd
        trace = bool(int(os.environ.get("AGNN_TRACE", "0")))
        tmpdir = os.environ.get("AGNN_TRACE_DIR") or None
        res = run_bass_kernel_spmd(nc, in_maps, core_ids=list(range(N_CORES)),
                                   trace=trace, tmpdir=tmpdir)
        results = res.results
        global LAST_RESULT
        LAST_RESULT = res

    out = np.zeros((n_nodes, d_feat), dtype=np.float32)
    node_of = plan["node_of"]
    for c in range(N_CORES):
        nd = node_of[c]
        valid = nd >= 0
        out[nd[valid]] = results[c]["out"][:slots][valid]
    return out
